# revision 3
# baseline (speedup 1.0000x reference)
"""GPTNeoX attention (B=1, S=2048, E=1024, 16 heads, hs=64) on 8 TRN2 cores.

Sharding: tensor-parallel across heads, 2 heads per core.
 - The matmul-rotary on q is folded into W_q on device:
     q_rot = x @ (W_q.T @ rotary) + b_q @ rotary
   so each core only ever materializes its own 128 q/k/v columns.
 - Attention is computed in transposed score layout ST[sk, sq] = (K Q^T),
   P~ = exp(ST/8) without max subtraction (scores are ~N(0, 0.26) for this
   model scale, exp never overflows), and the softmax denominator comes for
   free from a ones-column appended to V in the PV matmul.
 - Each core emits a partial output projection (its 128 y columns x W_dense
   slice); the host sums the 8 partials and adds b_dense (the unshard step).
"""

import os
import numpy as np

import concourse.bass as bass
import concourse.mybir as mybir
import concourse.tile as tile
from concourse import bacc
from concourse.bass_utils import run_bass_kernel_spmd
from concourse.masks import make_identity

FP = mybir.dt.float32
AF = mybir.ActivationFunctionType

N_CORES = 8
E = 1024          # embed dim
S = 2048          # sequence
P = 128           # partitions
EO = E // P       # 8 e-chunks
HS = 64           # head size
NH_LOC = 2        # heads per core
SQB = 1024        # sq block
NSQB = S // SQB   # 2
SKC = S // P      # 16 sk chunks
NSC = S // P      # 16 s chunks for output


def build_nc():
    nc = bacc.Bacc("TRN2", target_bir_lowering=False, debug=False)

    xT_d = nc.dram_tensor("xT", (E, S), FP, kind="ExternalInput")
    wq_d = nc.dram_tensor("wq", (E, E), FP, kind="ExternalInput")
    rot_d = nc.dram_tensor("rot", (E, P), FP, kind="ExternalInput")
    wkT_d = nc.dram_tensor("wkT", (E, P), FP, kind="ExternalInput")
    wvT_d = nc.dram_tensor("wvT", (E, P), FP, kind="ExternalInput")
    wdT_d = nc.dram_tensor("wdT", (P, E), FP, kind="ExternalInput")
    bq_d = nc.dram_tensor("bq", (E,), FP, kind="ExternalInput")
    bk_d = nc.dram_tensor("bk", (P,), FP, kind="ExternalInput")
    bv_d = nc.dram_tensor("bv", (P,), FP, kind="ExternalInput")
    out_d = nc.dram_tensor("out", (S, E), FP, kind="ExternalOutput")

    xT_r = xT_d[:].rearrange("(eo p) s -> p eo s", p=P)
    wq_r = wq_d[:].rearrange("(fo p) e -> p fo e", p=P)
    rot_r = rot_d[:].rearrange("(fo p) g -> p fo g", p=P)
    wkT_r = wkT_d[:].rearrange("(eo p) g -> p eo g", p=P)
    wvT_r = wvT_d[:].rearrange("(eo p) g -> p eo g", p=P)
    bq_r = bq_d[:].rearrange("(fo p) -> p fo", p=P)

    with tile.TileContext(nc) as tc:
        with (
            tc.tile_pool(name="const", bufs=1) as const,
            tc.tile_pool(name="wqc", bufs=3) as wqc,
            tc.tile_pool(name="work", bufs=3) as work,
            tc.tile_pool(name="outp", bufs=3) as outp,
            tc.tile_pool(name="psum", bufs=4, space="PSUM") as psum,
        ):
            # ---------- constant loads ----------
            xT_sb = const.tile([P, EO, S], FP)
            for eo in range(EO):
                nc.sync.dma_start(xT_sb[:, eo, :], xT_r[:, eo, :])
            rot_sb = const.tile([P, EO, P], FP)
            nc.sync.dma_start(rot_sb[:], rot_r[:])
            wkT_sb = const.tile([P, EO, P], FP)
            nc.sync.dma_start(wkT_sb[:], wkT_r[:])
            wvT_sb = const.tile([P, EO, P], FP)
            nc.sync.dma_start(wvT_sb[:], wvT_r[:])
            wdT_sb = const.tile([P, E], FP)
            nc.sync.dma_start(wdT_sb[:], wdT_d[:])
            bq_sb = const.tile([P, EO], FP)
            nc.sync.dma_start(bq_sb[:], bq_r[:])
            bk_sb = const.tile([P, 1], FP)
            nc.sync.dma_start(bk_sb[:], bk_d[:][:, None])
            bv_sb = const.tile([P, 1], FP)
            nc.sync.dma_start(bv_sb[:], bv_d[:][:, None])
            ident_sb = const.tile([P, P], FP)
            make_identity(nc, ident_sb[:])
            ones_sb = const.tile([1, HS], FP)
            nc.gpsimd.memset(ones_sb[:], 1.0)

            # ---------- fold rotary into W_q ----------
            # wqEff[g, e] = sum_f rot[f, g] * W_q[f, e], then transpose to
            # wqT[e, g] chunks (the lhsT layout the q projection needs).
            wqT_sb = const.tile([P, EO, P], FP)
            wqEff_sb = const.tile([P, E], FP)
            ps_fold = psum.tile([P, SQB], FP, tag="ps")
            for fo in range(EO):
                wq_chunk = wqc.tile([P, E], FP, tag="wq")
                nc.sync.dma_start(wq_chunk[:], wq_r[:, fo, :])
                for nn in range(E // 512):
                    nc.tensor.matmul(
                        ps_fold[:, nn * 512:(nn + 1) * 512],
                        lhsT=rot_sb[:, fo, :],
                        rhs=wq_chunk[:, nn * 512:(nn + 1) * 512],
                        start=(fo == 0),
                        stop=(fo == EO - 1),
                    )
            nc.vector.tensor_copy(wqEff_sb[:], ps_fold[:])
            for ec in range(EO):
                pst = psum.tile([P, SQB], FP, tag="ps")
                nc.tensor.transpose(
                    pst[:, :P], wqEff_sb[:, ec * P:(ec + 1) * P], ident_sb[:]
                )
                nc.vector.tensor_copy(wqT_sb[:, ec, :], pst[:, :P])

            # bqe[g] = sum_f b_q[f] * rot[f, g]
            bqe_sb = const.tile([P, 1], FP)
            ps_bq = psum.tile([P, SQB], FP, tag="ps")
            for fo in range(EO):
                nc.tensor.matmul(
                    ps_bq[:, :1],
                    lhsT=rot_sb[:, fo, :],
                    rhs=bq_sb[:, fo:fo + 1],
                    start=(fo == 0),
                    stop=(fo == EO - 1),
                )
            nc.vector.tensor_copy(bqe_sb[:], ps_bq[:, :1])

            # ---------- q/k/v projections (transposed layouts) ----------
            # qT[g, s] = sum_e wqT[e, g] xT[e, s] + bqe[g]
            qT_sb = const.tile([P, S], FP)
            kT_sb = const.tile([P, S], FP)
            vT_sb = const.tile([P, S], FP)
            for (dst, w, bias_ap) in (
                (kT_sb, wkT_sb, bk_sb),
                (vT_sb, wvT_sb, None),
                (qT_sb, wqT_sb, bqe_sb),
            ):
                for sb in range(S // SQB):
                    ps = psum.tile([P, SQB], FP, tag="ps")
                    for ec in range(EO):
                        for nn in range(SQB // 512):
                            nc.tensor.matmul(
                                ps[:, nn * 512:(nn + 1) * 512],
                                lhsT=w[:, ec, :],
                                rhs=xT_sb[:, ec,
                                          sb * SQB + nn * 512:
                                          sb * SQB + (nn + 1) * 512],
                                start=(ec == 0),
                                stop=(ec == EO - 1),
                            )
                    dslice = dst[:, sb * SQB:(sb + 1) * SQB]
                    if bias_ap is None:
                        nc.scalar.copy(dslice, ps[:])
                    else:
                        nc.scalar.add(dslice, ps[:], bias_ap[:])

            # ---------- V in [sk, d] layout (+ ones column) ----------
            vaug_sb = const.tile([P, NH_LOC, SKC, HS + 1], FP)
            nc.gpsimd.memset(vaug_sb[:, :, :, HS:HS + 1], 1.0)
            for h in range(NH_LOC):
                for j in range(SKC):
                    ps = psum.tile([P, SQB], FP, tag="ps")
                    nc.tensor.transpose(
                        ps[:, :HS],
                        vT_sb[h * HS:(h + 1) * HS, j * P:(j + 1) * P],
                        ident_sb[h * HS:(h + 1) * HS, h * HS:(h + 1) * HS],
                    )
                    nc.vector.tensor_copy(vaug_sb[:, h, j, :HS], ps[:, :HS])

            # ---------- attention ----------
            # ST[sk, sq] = K Q^T (per head);  P~ = exp(ST/8)
            # yT_aug[d|Z, sq] = [V | 1]^T P~
            yTn_sb = const.tile([P, S], FP)
            for h in range(NH_LOC):
                hsl = slice(h * HS, (h + 1) * HS)
                for qb in range(NSQB):
                    qsl = slice(qb * SQB, (qb + 1) * SQB)
                    yt = psum.tile([P, SQB], FP, tag="ps")
                    for j in range(SKC):
                        st = psum.tile([P, SQB], FP, tag="ps")
                        for nn in range(SQB // 512):
                            nsl = slice(nn * 512, (nn + 1) * 512)
                            nc.tensor.matmul(
                                st[:, nsl],
                                lhsT=kT_sb[hsl, j * P:(j + 1) * P],
                                rhs=qT_sb[hsl, qb * SQB + nn * 512:
                                          qb * SQB + (nn + 1) * 512],
                                start=True,
                                stop=True,
                            )
                        pt = work.tile([P, SQB], FP, tag="pt")
                        nc.scalar.activation(pt[:], st[:], AF.Exp, scale=0.125)
                        for nn in range(SQB // 512):
                            nsl = slice(nn * 512, (nn + 1) * 512)
                            nc.tensor.matmul(
                                yt[:HS + 1, nsl],
                                lhsT=vaug_sb[:, h, j, :],
                                rhs=pt[:, nsl],
                                start=(j == 0),
                                stop=(j == SKC - 1),
                            )
                    # normalize: y = yT[:HS] / Z + b_v ; Z in row HS
                    zr = work.tile([1, SQB], FP, tag="zr")
                    nc.vector.reciprocal(zr[:], yt[HS:HS + 1, :])
                    zb = psum.tile([P, SQB], FP, tag="ps")
                    for nn in range(SQB // 512):
                        nsl = slice(nn * 512, (nn + 1) * 512)
                        nc.tensor.matmul(
                            zb[:HS, nsl],
                            lhsT=ones_sb[:],
                            rhs=zr[:, nsl],
                            start=True,
                            stop=True,
                        )
                    zbs = work.tile([HS, SQB], FP, tag="zbs")
                    nc.vector.tensor_copy(zbs[:], zb[:HS, :])
                    ysl = yTn_sb[hsl, qsl]
                    nc.vector.tensor_mul(ysl, yt[:HS, :], zbs[:])
                    nc.vector.tensor_scalar_add(ysl, ysl, bv_sb[hsl, :])

            # ---------- partial output projection ----------
            # out[s, f] = sum_e yTn[e, s] wdT[e, f]
            for sc in range(NSC):
                po = psum.tile([P, SQB], FP, tag="ps")
                for nn in range(E // 512):
                    nsl = slice(nn * 512, (nn + 1) * 512)
                    nc.tensor.matmul(
                        po[:, nsl],
                        lhsT=yTn_sb[:, sc * P:(sc + 1) * P],
                        rhs=wdT_sb[:, nsl],
                        start=True,
                        stop=True,
                    )
                ob = outp.tile([P, E], FP, tag="ob")
                nc.vector.tensor_copy(ob[:], po[:])
                nc.sync.dma_start(out_d[sc * P:(sc + 1) * P, :], ob[:])

    nc.compile()
    return nc


_NC_CACHE = None


def _get_nc():
    global _NC_CACHE
    if _NC_CACHE is None:
        _NC_CACHE = build_nc()
    return _NC_CACHE


def make_in_maps(x, W_qkv, b_qkv, rotary, W_dense, b_dense):
    x = np.asarray(x, dtype=np.float32)
    W_qkv = np.asarray(W_qkv, dtype=np.float32)
    b_qkv = np.asarray(b_qkv, dtype=np.float32)
    rotary = np.asarray(rotary, dtype=np.float32)
    W_dense = np.asarray(W_dense, dtype=np.float32)

    xT = np.ascontiguousarray(x.reshape(S, E).T)
    wq = np.ascontiguousarray(W_qkv[0:E, :])
    bq = np.ascontiguousarray(b_qkv[0:E])
    in_maps = []
    for c in range(N_CORES):
        lo, hi = P * c, P * (c + 1)
        in_maps.append({
            "xT": xT,
            "wq": wq,
            "rot": np.ascontiguousarray(rotary[:, lo:hi]),
            "wkT": np.ascontiguousarray(W_qkv[E + lo:E + hi, :].T),
            "wvT": np.ascontiguousarray(W_qkv[2 * E + lo:2 * E + hi, :].T),
            "wdT": np.ascontiguousarray(W_dense[:, lo:hi].T),
            "bq": bq,
            "bk": np.ascontiguousarray(b_qkv[E + lo:E + hi]),
            "bv": np.ascontiguousarray(b_qkv[2 * E + lo:2 * E + hi]),
        })
    return in_maps


def run(inputs, trace=False, **trace_kwargs):
    """Run on 8 cores; returns (full_output, BassKernelResults)."""
    nc = _get_nc()
    in_maps = make_in_maps(**inputs)
    br = run_bass_kernel_spmd(
        nc, in_maps, core_ids=list(range(N_CORES)), trace=trace, **trace_kwargs
    )
    b_dense = np.asarray(inputs["b_dense"], dtype=np.float32)
    acc = np.zeros((S, E), dtype=np.float32)
    for r in br.results:
        acc += np.asarray(r["out"], dtype=np.float32)
    acc += b_dense[None, :]
    return acc[None, :, :], br


def kernel(**inputs) -> np.ndarray:
    out, _ = run(inputs, trace=False)
    return out
